# revision 4
# baseline (speedup 1.0000x reference)
"""Bass/Tile Trainium2 kernel for nn_Attention_7284264534326.

Single-head attention, B=8, S=2048, D=1024:
    q = (x1 @ wq) * D**-0.5 ; k = x2 @ wk ; v = x2 @ wv
    a = softmax(q @ k^T + mask * -1e9, axis=-1)
    out = relu(a @ v) @ wo

Sharding: data-parallel over batch; one batch element per NeuronCore (8 cores).
Each core runs the identical program on its own x1[b], x2[b], maskSeq[b].

Per-core dataflow (all matmul operands bf16, PSUM accumulation f32):
  - x1, x2 are PE-transposed into X1T/X2T ([d, s] layout, bf16).
  - qT[e,s] = wq^T-free matmul(lhsT=wq[d,e], rhs=X1T[d,s]) with 1/32 scale on
    PSUM evacuation; kT likewise; V[s,e] = matmul(lhsT=X2T[d,s], rhs=wv[d,e]).
  - scores^T[k,q] = matmul(lhsT=kT, rhs=qT); exp fused into ACT evacuation
    with the additive mask as a per-partition bias (exp(-1e9) == 0).
  - yU^T[e,q] = matmul(lhsT=V, rhs=exp^T); denom via matmul with ones-vector.
  - z^T = relu(yU^T) (normalization deferred: relu(y/d) == relu(y)/d for d>0).
  - out[q,f] = matmul(lhsT=z^T, rhs=wo) scaled by 1/denom on evacuation.
"""

import numpy as np
from contextlib import ExitStack

B, S, D = 8, 2048, 1024
P = 128
DC = D // P       # 8 chunks of the depth/contraction dim
EC = D // P       # 8 chunks of the embedding dim
SC = S // P       # 16 chunks of the sequence dim
Q_TILE = 256      # queries per attention tile
NQT = S // Q_TILE # 8
S_SLICE = 512     # seq rows per projection slice
NSL = S // S_SLICE
N_CORES = 8
QSCALE = float(D) ** -0.5  # 1/32

_cached_nc = None


def _build():
    import concourse.tile as tile
    from concourse import bacc, mybir
    from concourse.masks import make_identity

    f32 = mybir.dt.float32
    bf16 = mybir.dt.bfloat16
    i32 = mybir.dt.int32
    AF = mybir.ActivationFunctionType

    nc = bacc.Bacc("TRN2", target_bir_lowering=False, debug=False,
                   enable_asserts=False, num_devices=N_CORES)

    x1 = nc.dram_tensor("x1", [S, D], f32, kind="ExternalInput").ap()
    x2 = nc.dram_tensor("x2", [S, D], f32, kind="ExternalInput").ap()
    msk = nc.dram_tensor("msk", [1, S], i32, kind="ExternalInput").ap()
    wq = nc.dram_tensor("wq", [D, D], f32, kind="ExternalInput").ap()
    wk = nc.dram_tensor("wk", [D, D], f32, kind="ExternalInput").ap()
    wv = nc.dram_tensor("wv", [D, D], f32, kind="ExternalInput").ap()
    wo = nc.dram_tensor("wo", [D, D], f32, kind="ExternalInput").ap()
    out = nc.dram_tensor("out", [S, D], f32, kind="ExternalOutput").ap()

    with tile.TileContext(nc) as tc, ExitStack() as ctx:
        persist = ctx.enter_context(tc.tile_pool(name="persist", bufs=1))

        qT = persist.tile([P, EC, S], bf16, name="qT")        # [e, s] by e-chunk
        kT = persist.tile([P, EC, S], bf16, name="kT")
        V = persist.tile([P, SC, D], bf16, name="V")          # [s, e] by s-chunk
        wo_bf = persist.tile([P, DC, D], bf16, name="wo_bf")
        ident = persist.tile([P, P], f32, name="ident")
        ones_bf = persist.tile([P, 1], bf16, name="ones_bf")
        maskbias = persist.tile([P, SC], f32, name="maskbias")

        make_identity(nc, ident)
        nc.vector.memset(ones_bf, 1.0)

        # ---- mask -> per-partition additive bias, [s] laid out as [P, SC] ----
        with tc.tile_pool(name="mprep", bufs=1) as mprep, \
             tc.tile_pool(name="mpsum", bufs=1, space="PSUM") as mpsum:
            mint = mprep.tile([SC, P], i32, name="mint")
            nc.sync.dma_start(mint, msk.rearrange("o (c p) -> (o c) p", p=P))
            mf = mprep.tile([SC, P], f32, name="mf")
            nc.vector.tensor_copy(out=mf, in_=mint)           # int32 -> f32
            mneg = mprep.tile([SC, P], f32, name="mneg")
            nc.scalar.mul(mneg, mf, -1.0e9)
            mp = mpsum.tile([P, SC], f32, name="mp")
            nc.tensor.transpose(mp, mneg, ident[:SC, :SC])    # [SC,P] -> [P,SC]
            nc.vector.tensor_copy(out=maskbias, in_=mp)

        # ================= phase 1: projections =================
        with ExitStack() as pctx:
            wpool = pctx.enter_context(tc.tile_pool(name="wpool", bufs=1))
            wstage = pctx.enter_context(tc.tile_pool(name="wstage", bufs=2))
            xstage = pctx.enter_context(tc.tile_pool(name="xstage", bufs=2))
            xtpool = pctx.enter_context(tc.tile_pool(name="xtpool", bufs=1))
            tpsum = pctx.enter_context(tc.tile_pool(name="tpsum", bufs=4, space="PSUM"))
            ppsum = pctx.enter_context(tc.tile_pool(name="ppsum", bufs=4, space="PSUM"))

            wq_bf = wpool.tile([P, DC, D], bf16, name="wq_bf")
            wk_bf = wpool.tile([P, DC, D], bf16, name="wk_bf")
            wv_bf = wpool.tile([P, DC, D], bf16, name="wv_bf")
            for w_ap, w_bf, wnm in ((wq, wq_bf, "q"), (wk, wk_bf, "k"), (wv, wv_bf, "v")):
                for dc in range(DC):
                    ws = wstage.tile([P, D], f32, name="ws", tag="ws")
                    nc.sync.dma_start(ws, w_ap[dc * P:(dc + 1) * P, :])
                    nc.vector.tensor_copy(out=w_bf[:, dc, :], in_=ws)

            for sl in range(NSL):
                s0 = sl * S_SLICE
                x1t = xtpool.tile([P, DC, S_SLICE], bf16, name="x1t", tag="x1t")
                x2t = xtpool.tile([P, DC, S_SLICE], bf16, name="x2t", tag="x2t")
                for j in range(S_SLICE // P):
                    for x_ap, xt, nm in ((x1, x1t, "x1s"), (x2, x2t, "x2s")):
                        xs = xstage.tile([P, D], f32, name=nm, tag=nm)
                        nc.sync.dma_start(xs, x_ap[s0 + j * P: s0 + (j + 1) * P, :])
                        for dc in range(DC):
                            pt = tpsum.tile([P, P], f32, name="pt", tag="pt")
                            nc.tensor.transpose(pt, xs[:, dc * P:(dc + 1) * P], ident)
                            nc.vector.tensor_copy(
                                out=xt[:, dc, j * P:(j + 1) * P], in_=pt)

                # qT / kT for this slice
                for w_bf, dstT, scale in ((wq_bf, qT, QSCALE), (wk_bf, kT, None)):
                    for ec in range(EC):
                        pq = ppsum.tile([P, S_SLICE], f32, name="pq", tag="pp")
                        src = x1t if dstT is qT else x2t
                        for dc in range(DC):
                            nc.tensor.matmul(
                                pq, lhsT=w_bf[:, dc, ec * P:(ec + 1) * P],
                                rhs=src[:, dc, :],
                                start=(dc == 0), stop=(dc == DC - 1))
                        if scale is not None:
                            nc.scalar.activation(
                                out=dstT[:, ec, s0:s0 + S_SLICE], in_=pq,
                                func=AF.Copy, scale=scale)
                        else:
                            nc.scalar.activation(
                                out=dstT[:, ec, s0:s0 + S_SLICE], in_=pq,
                                func=AF.Copy)

                # V rows for this slice
                for j in range(S_SLICE // P):
                    for eh in range(2):
                        pv = ppsum.tile([P, 512], f32, name="pv", tag="pp")
                        for dc in range(DC):
                            nc.tensor.matmul(
                                pv, lhsT=x2t[:, dc, j * P:(j + 1) * P],
                                rhs=wv_bf[:, dc, eh * 512:(eh + 1) * 512],
                                start=(dc == 0), stop=(dc == DC - 1))
                        nc.vector.tensor_copy(
                            out=V[:, sl * (S_SLICE // P) + j, eh * 512:(eh + 1) * 512],
                            in_=pv)

        # wo cast (attention-phase resident)
        with tc.tile_pool(name="wstage2", bufs=2) as wstage2:
            for dc in range(DC):
                ws2 = wstage2.tile([P, D], f32, name="ws2", tag="ws2")
                nc.sync.dma_start(ws2, wo[dc * P:(dc + 1) * P, :])
                nc.vector.tensor_copy(out=wo_bf[:, dc, :], in_=ws2)

        # ================= phase 2: attention =================
        epool = ctx.enter_context(tc.tile_pool(name="epool", bufs=2))
        zpool = ctx.enter_context(tc.tile_pool(name="zpool", bufs=2))
        opool = ctx.enter_context(tc.tile_pool(name="opool", bufs=3))
        rpool = ctx.enter_context(tc.tile_pool(name="rpool", bufs=2))
        spsum = ctx.enter_context(tc.tile_pool(name="spsum", bufs=2, space="PSUM"))
        ypsum = ctx.enter_context(tc.tile_pool(name="ypsum", bufs=3, space="PSUM"))
        dpsum = ctx.enter_context(tc.tile_pool(name="dpsum", bufs=1, space="PSUM"))
        opsum = ctx.enter_context(tc.tile_pool(name="opsum", bufs=2, space="PSUM"))

        for qt in range(NQT):
            q0 = qt * Q_TILE
            expt = epool.tile([P, SC, Q_TILE], bf16, name="expt", tag="expt")
            for kc in range(SC):
                ps = spsum.tile([P, Q_TILE], f32, name="ps", tag="ps")
                for ec in range(EC):
                    nc.tensor.matmul(
                        ps, lhsT=kT[:, ec, kc * P:(kc + 1) * P],
                        rhs=qT[:, ec, q0:q0 + Q_TILE],
                        start=(ec == 0), stop=(ec == EC - 1))
                nc.scalar.activation(
                    out=expt[:, kc, :], in_=ps, func=AF.Exp,
                    bias=maskbias[:, kc:kc + 1], scale=1.0)

            # NOTE: matmul start=True clears has_written bits for the WHOLE
            # PSUM bank, so accumulation chains must not interleave within a
            # bank: run one chain per psum tile to completion before starting
            # the next chain that shares its bank.
            zt = zpool.tile([P, EC, Q_TILE], bf16, name="zt", tag="zt")
            for ec in range(EC):
                py = ypsum.tile([P, Q_TILE], f32, name="py", tag="py")
                for kc in range(SC):
                    nc.tensor.matmul(
                        py, lhsT=V[:, kc, ec * P:(ec + 1) * P],
                        rhs=expt[:, kc, :],
                        start=(kc == 0), stop=(kc == SC - 1))
                nc.scalar.activation(out=zt[:, ec, :], in_=py, func=AF.Relu)

            pd = dpsum.tile([P, Q_TILE // P], f32, name="pd", tag="pd")
            for qs in range(Q_TILE // P):
                for kc in range(SC):
                    nc.tensor.matmul(
                        pd[:, qs:qs + 1],
                        lhsT=expt[:, kc, qs * P:(qs + 1) * P], rhs=ones_bf,
                        start=(kc == 0), stop=(kc == SC - 1))
            recip = rpool.tile([P, Q_TILE // P], f32, name="recip", tag="recip")
            nc.vector.reciprocal(recip, pd)

            for qs in range(Q_TILE // P):
                osb = opool.tile([P, D], f32, name="osb", tag="osb")
                for fh in range(2):
                    po = opsum.tile([P, 512], f32, name="po", tag="po")
                    for ec in range(EC):
                        nc.tensor.matmul(
                            po, lhsT=zt[:, ec, qs * P:(qs + 1) * P],
                            rhs=wo_bf[:, ec, fh * 512:(fh + 1) * 512],
                            start=(ec == 0), stop=(ec == EC - 1))
                    nc.scalar.activation(
                        out=osb[:, fh * 512:(fh + 1) * 512], in_=po,
                        func=AF.Copy, scale=recip[:, qs:qs + 1])
                nc.sync.dma_start(out[q0 + qs * P: q0 + (qs + 1) * P, :], osb)

    nc.compile()
    return nc


def kernel(x1, x2, maskSeq, wq, wk, wv, wo, **_unused):
    from concourse.bass_utils import run_bass_kernel_spmd

    global _cached_nc
    if _cached_nc is None:
        _cached_nc = _build()
    nc = _cached_nc

    x1 = np.ascontiguousarray(np.asarray(x1, dtype=np.float32))
    x2 = np.ascontiguousarray(np.asarray(x2, dtype=np.float32))
    maskSeq = np.ascontiguousarray(np.asarray(maskSeq, dtype=np.int32))
    wq = np.ascontiguousarray(np.asarray(wq, dtype=np.float32))
    wk = np.ascontiguousarray(np.asarray(wk, dtype=np.float32))
    wv = np.ascontiguousarray(np.asarray(wv, dtype=np.float32))
    wo = np.ascontiguousarray(np.asarray(wo, dtype=np.float32))

    in_maps = [
        {"x1": x1[c], "x2": x2[c], "msk": maskSeq[c],
         "wq": wq, "wk": wk, "wv": wv, "wo": wo}
        for c in range(N_CORES)
    ]
    res = run_bass_kernel_spmd(nc, in_maps, core_ids=list(range(N_CORES)))
    return np.stack([res.results[c]["out"] for c in range(N_CORES)], axis=0)
